# revision 7
# baseline (speedup 1.0000x reference)
"""Trainium2 Bass kernel for ContextHyperLinearSSM.

Computes out[b,:] = x[b,:] @ (WA[context[b]] * adj_xx) + u[b,:] @ (WB[context[b]] * adj_xu)

Strategy: shard the CONTEXT axis across the 8 cores (64 contexts each).
The host groups samples by context (padded to the max group size G), so each
core streams its 64 contexts' weight banks from HBM exactly once, applies the
adjacency masks on-device, and runs 3 accumulating matmuls per context
(two K=128 chunks of the A term + one for the B term).  Each sample's row is
computed by exactly one core, so the host-side unshard is a pure scatter.
"""

import sys

sys.path.insert(0, "/opt/trn_rl_repo")

import numpy as np

import concourse.bass as bass
import concourse.mybir as mybir
import concourse.tile as tile
from concourse import bacc
from concourse.bass import ts
from concourse.bass_utils import run_bass_kernel_spmd

N_CORES = 8
CT = 8  # contexts per DMA group

# matmul operand dtype: float32 (4 cyc/row) or float32r (1 cyc/row at N>=256)
MM_DT = mybir.dt.float32


def _install_profile_shim():
    """Register the NTFF profile hook that trn_boot skips when
    antenv.axon_hooks is missing from the image (profiling only)."""
    import types
    if "antenv.axon_hooks" in sys.modules:
        return
    try:
        from trn_agent_boot.trn_boot import _ntff_profile_via_ctypes
        hook = _ntff_profile_via_ctypes("/opt/axon/libaxon_pjrt.so")
    except Exception:
        hook = None
    mod = types.ModuleType("antenv.axon_hooks")
    mod.get_axon_ntff_profile_hook = lambda: hook
    mod.set_axon_ntff_profile_hook = lambda h: None
    sys.modules["antenv.axon_hooks"] = mod


def _build_program(CP, S, A, G):
    """Build the per-core Bass program. CP contexts/core, group size G."""
    f32 = mybir.dt.float32
    nc = bacc.Bacc("TRN2", target_bir_lowering=False)

    HS = S // 128  # number of 128-row K-chunks of the A-term contraction
    assert S % 128 == 0 and A == 128

    wa = nc.dram_tensor("wa", [CP, HS, 128, S], f32, kind="ExternalInput").ap()
    wb = nc.dram_tensor("wb", [CP, A, S], f32, kind="ExternalInput").ap()
    xt = nc.dram_tensor("xt", [CP, HS, 128, G], f32, kind="ExternalInput").ap()
    ut = nc.dram_tensor("ut", [CP, A, G], f32, kind="ExternalInput").ap()
    adj_xx = nc.dram_tensor("adj_xx", [HS, 128, S], mybir.dt.uint8,
                            kind="ExternalInput").ap()
    adj_xu = nc.dram_tensor("adj_xu", [A, S], mybir.dt.uint8,
                            kind="ExternalInput").ap()
    out = nc.dram_tensor("out", [CP, G, S], f32, kind="ExternalOutput").ap()

    NG = CP // CT
    assert CP % CT == 0

    with tile.TileContext(nc) as tc:
        with (
            tc.tile_pool(name="const", bufs=1) as const,
            tc.tile_pool(name="w", bufs=3) as wpool,
            tc.tile_pool(name="xu", bufs=3) as xpool,
            tc.tile_pool(name="o", bufs=3) as opool,
            tc.tile_pool(name="psum", bufs=8, space="PSUM") as psum,
        ):
            # adjacency masks, cast uint8 -> f32 during the (SWDGE) DMA
            adjA0 = const.tile([128, HS, S], f32)
            nc.gpsimd.dma_start(adjA0[:], adj_xx.rearrange("h p s -> p h s"))
            adjB0 = const.tile([128, S], f32)
            nc.gpsimd.dma_start(adjB0[:], adj_xu[:])
            # funnel through VectorE so the per-group mask-multiplies get
            # same-engine deps on the masks (no extra semaphore waits)
            adjA = const.tile([128, HS, S], f32)
            nc.vector.tensor_copy(adjA[:], adjA0[:])
            adjB = const.tile([128, S], f32)
            nc.vector.tensor_copy(adjB[:], adjB0[:])

            for g in range(NG):
                cs = ts(g, CT)
                wa_t = wpool.tile([128, CT, HS, S], f32)
                nc.sync.dma_start(wa_t[:], wa[cs].rearrange("c h p s -> p c h s"))
                wb_t = wpool.tile([128, CT, S], f32)
                nc.sync.dma_start(wb_t[:], wb[cs].rearrange("c p s -> p c s"))
                xt_t = xpool.tile([128, CT, HS, G], f32)
                nc.sync.dma_start(xt_t[:], xt[cs].rearrange("c h p g -> p c h g"))
                ut_t = xpool.tile([128, CT, G], f32)
                nc.sync.dma_start(ut_t[:], ut[cs].rearrange("c p g -> p c g"))

                # mask the weights in place
                nc.vector.tensor_tensor(
                    wa_t[:], wa_t[:],
                    adjA[:, None, :, :].to_broadcast([128, CT, HS, S]),
                    mybir.AluOpType.mult)
                nc.vector.tensor_tensor(
                    wb_t[:], wb_t[:],
                    adjB[:, None, :].to_broadcast([128, CT, S]),
                    mybir.AluOpType.mult)

                out_sb = opool.tile([min(G, 128), CT, S], f32)
                for c in range(CT):
                    for m0 in range(0, G, 128):
                        mg = min(128, G - m0)
                        ps = psum.tile([mg, S], f32)
                        for h in range(HS):
                            nc.tensor.matmul(
                                ps[:],
                                lhsT=xt_t[:, c, h, m0:m0 + mg].bitcast(MM_DT),
                                rhs=wa_t[:, c, h, :].bitcast(MM_DT),
                                start=(h == 0), stop=False)
                        nc.tensor.matmul(
                            ps[:],
                            lhsT=ut_t[:, c, m0:m0 + mg].bitcast(MM_DT),
                            rhs=wb_t[:, c, :].bitcast(MM_DT),
                            start=False, stop=True)
                        if m0 == 0:
                            nc.scalar.copy(out_sb[:mg, c, :], ps[:])
                        else:
                            # rare G>128 spill: DMA the extra chunk directly
                            nc.scalar.dma_start(
                                out[g * CT + c, m0:m0 + mg, :], ps[:])
                nc.scalar.dma_start(
                    out[cs].rearrange("c g s -> g c s"), out_sb[:])

    nc.compile()
    return nc


def kernel(x, u, WA, WB, adj_xx, adj_xu, context, _trace=False):
    B, S = x.shape
    _, A = u.shape
    C = WA.shape[0]
    assert C % N_CORES == 0
    CP = C // N_CORES

    # ---- host-side shard: group samples by context --------------------
    context = np.asarray(context)
    cnt = np.bincount(context, minlength=C)
    G = int(cnt.max())
    G = max(4, ((G + 3) // 4) * 4)
    order = np.argsort(context, kind="stable")
    starts = np.zeros(C + 1, np.int64)
    starts[1:] = np.cumsum(cnt)
    j = np.arange(G)
    valid = j[None, :] < cnt[:, None]                      # [C, G]
    pos = starts[:-1, None] + np.minimum(j[None, :],
                                         np.maximum(cnt[:, None] - 1, 0))
    gidx = order[pos]                                      # [C, G]

    Xp = np.asarray(x, np.float32)[gidx]                   # [C, G, S]
    Up = np.asarray(u, np.float32)[gidx]                   # [C, G, A]
    XpT = np.ascontiguousarray(Xp.transpose(0, 2, 1))      # [C, S, G]
    UpT = np.ascontiguousarray(Up.transpose(0, 2, 1))      # [C, A, G]

    WA = np.ascontiguousarray(WA, np.float32)
    WB = np.ascontiguousarray(WB, np.float32)
    adjxx_u8 = np.ascontiguousarray(adj_xx).view(np.uint8).reshape(S // 128, 128, S)
    adjxu_u8 = np.ascontiguousarray(adj_xu).view(np.uint8)

    in_maps = []
    for k in range(N_CORES):
        sl = slice(k * CP, (k + 1) * CP)
        in_maps.append({
            "wa": WA[sl].reshape(CP, S // 128, 128, S),
            "wb": WB[sl],
            "xt": XpT[sl].reshape(CP, S // 128, 128, G),
            "ut": UpT[sl],
            "adj_xx": adjxx_u8,
            "adj_xu": adjxu_u8,
        })

    if _trace:
        _install_profile_shim()
    nc = _build_program(CP, S, A, G)
    res = run_bass_kernel_spmd(nc, in_maps, core_ids=list(range(N_CORES)),
                               trace=_trace)

    Out_all = np.concatenate([r["out"] for r in res.results], axis=0)  # [C,G,S]
    out_full = np.zeros((B, S), np.float32)
    out_full[gidx[valid]] = Out_all[valid]

    if _trace:
        return out_full, res
    return out_full


# revision 10
# speedup vs baseline: 1.0629x; 1.0629x over previous
"""Trainium2 Bass kernel for ContextHyperLinearSSM.

Computes out[b,:] = x[b,:] @ (WA[context[b]] * adj_xx) + u[b,:] @ (WB[context[b]] * adj_xu)

Strategy: shard the CONTEXT axis across the 8 cores (64 contexts each).
The host groups samples by context (padded to the max group size G), so each
core streams its 64 contexts' weight banks from HBM exactly once, applies the
adjacency masks on-device, and runs 3 accumulating matmuls per context
(two K=128 chunks of the A term + one for the B term).  Each sample's row is
computed by exactly one core, so the host-side unshard is a pure scatter.
"""

import sys

sys.path.insert(0, "/opt/trn_rl_repo")

import numpy as np

import concourse.bass as bass
import concourse.mybir as mybir
import concourse.tile as tile
from concourse import bacc
from concourse.bass import ts
from concourse.bass_utils import run_bass_kernel_spmd

N_CORES = 8
CT = 8  # contexts per DMA group

# matmul operand dtype: float32 (4 cyc/row) or float32r (1 cyc/row at N>=256)
MM_DT = mybir.dt.float32r


def _install_profile_shim():
    """Register the NTFF profile hook that trn_boot skips when
    antenv.axon_hooks is missing from the image (profiling only)."""
    import types
    if "antenv.axon_hooks" in sys.modules:
        return
    try:
        from trn_agent_boot.trn_boot import _ntff_profile_via_ctypes
        hook = _ntff_profile_via_ctypes("/opt/axon/libaxon_pjrt.so")
    except Exception:
        hook = None
    mod = types.ModuleType("antenv.axon_hooks")
    mod.get_axon_ntff_profile_hook = lambda: hook
    mod.set_axon_ntff_profile_hook = lambda h: None
    sys.modules["antenv.axon_hooks"] = mod


def _build_program(CP, S, A, G):
    """Build the per-core Bass program. CP contexts/core, group size G."""
    f32 = mybir.dt.float32
    nc = bacc.Bacc("TRN2", target_bir_lowering=False)

    HS = S // 128  # number of 128-row K-chunks of the A-term contraction
    assert S % 128 == 0 and A == 128

    wa = nc.dram_tensor("wa", [CP, HS, 128, S], f32, kind="ExternalInput").ap()
    wb = nc.dram_tensor("wb", [CP, A, S], f32, kind="ExternalInput").ap()
    xt = nc.dram_tensor("xt", [CP, HS, 128, G], f32, kind="ExternalInput").ap()
    ut = nc.dram_tensor("ut", [CP, A, G], f32, kind="ExternalInput").ap()
    adj_xx = nc.dram_tensor("adj_xx", [HS, 128, S], mybir.dt.uint8,
                            kind="ExternalInput").ap()
    adj_xu = nc.dram_tensor("adj_xu", [A, S], mybir.dt.uint8,
                            kind="ExternalInput").ap()
    out = nc.dram_tensor("out", [CP, G, S], f32, kind="ExternalOutput").ap()

    NG = CP // CT
    assert CP % CT == 0

    with tile.TileContext(nc) as tc:
        with (
            tc.tile_pool(name="const", bufs=1) as const,
            tc.tile_pool(name="w", bufs=2) as wpool,
            tc.tile_pool(name="xu", bufs=3) as xpool,
            tc.tile_pool(name="o", bufs=3) as opool,
            tc.tile_pool(name="psum", bufs=8, space="PSUM") as psum,
        ):
            # adjacency masks, cast uint8 -> f32 during the (SWDGE) DMA
            adjA0 = const.tile([128, HS, S], f32)
            nc.gpsimd.dma_start(adjA0[:], adj_xx.rearrange("h p s -> p h s"))
            adjB0 = const.tile([128, S], f32)
            nc.gpsimd.dma_start(adjB0[:], adj_xu[:])
            # funnel through VectorE so the per-group mask-multiplies get
            # same-engine deps on the masks (no extra semaphore waits)
            adjA = const.tile([128, HS, S], f32)
            nc.vector.tensor_copy(adjA[:], adjA0[:])
            adjB = const.tile([128, S], f32)
            nc.vector.tensor_copy(adjB[:], adjB0[:])

            rounded = MM_DT == mybir.dt.float32r
            for g in range(NG):
                cs = ts(g, CT)
                wa_t = wpool.tile([128, CT, HS, S], f32)
                nc.sync.dma_start(wa_t[:], wa[cs].rearrange("c h p s -> p c h s"))
                wb_t = wpool.tile([128, CT, S], f32)
                nc.sync.dma_start(wb_t[:], wb[cs].rearrange("c p s -> p c s"))
                xt_t = xpool.tile([128, CT, HS, G], f32)
                nc.sync.dma_start(xt_t[:], xt[cs].rearrange("c h p g -> p c h g"))
                ut_t = xpool.tile([128, CT, G], f32)
                nc.sync.dma_start(ut_t[:], ut[cs].rearrange("c p g -> p c g"))

                if rounded:
                    # fp32r consumers need fp32r-rounded producers
                    wa_m = wpool.tile([128, CT, HS, S], MM_DT, tag="wa_m")
                    wb_m = wpool.tile([128, CT, S], MM_DT, tag="wb_m")
                    xt_m = xpool.tile([128, CT, HS, G], MM_DT, tag="xt_m")
                    ut_m = xpool.tile([128, CT, G], MM_DT, tag="ut_m")
                    nc.vector.tensor_copy(xt_m[:], xt_t[:])
                    nc.vector.tensor_copy(ut_m[:], ut_t[:])
                else:
                    wa_m, wb_m, xt_m, ut_m = wa_t, wb_t, xt_t, ut_t

                # mask the weights
                nc.vector.tensor_tensor(
                    wa_m[:], wa_t[:],
                    adjA[:, None, :, :].to_broadcast([128, CT, HS, S]),
                    mybir.AluOpType.mult)
                nc.vector.tensor_tensor(
                    wb_m[:], wb_t[:],
                    adjB[:, None, :].to_broadcast([128, CT, S]),
                    mybir.AluOpType.mult)

                out_sb = opool.tile([min(G, 128), CT, S], f32)
                for c in range(CT):
                    for m0 in range(0, G, 128):
                        mg = min(128, G - m0)
                        ps = psum.tile([mg, S], f32)
                        for h in range(HS):
                            nc.tensor.matmul(
                                ps[:],
                                lhsT=xt_m[:, c, h, m0:m0 + mg],
                                rhs=wa_m[:, c, h, :],
                                start=(h == 0), stop=False)
                        nc.tensor.matmul(
                            ps[:],
                            lhsT=ut_m[:, c, m0:m0 + mg],
                            rhs=wb_m[:, c, :],
                            start=False, stop=True)
                        if m0 == 0:
                            nc.scalar.copy(out_sb[:mg, c, :], ps[:])
                        else:
                            # rare G>128 spill: DMA the extra chunk directly
                            nc.scalar.dma_start(
                                out[g * CT + c, m0:m0 + mg, :], ps[:])
                nc.scalar.dma_start(
                    out[cs].rearrange("c g s -> g c s"), out_sb[:])

    nc.compile()
    return nc


def kernel(x, u, WA, WB, adj_xx, adj_xu, context, _trace=False):
    B, S = x.shape
    _, A = u.shape
    C = WA.shape[0]
    assert C % N_CORES == 0
    CP = C // N_CORES

    # ---- host-side shard: group samples by context --------------------
    context = np.asarray(context)
    cnt = np.bincount(context, minlength=C)
    G = int(cnt.max())
    G = max(4, ((G + 3) // 4) * 4)
    order = np.argsort(context, kind="stable")
    starts = np.zeros(C + 1, np.int64)
    starts[1:] = np.cumsum(cnt)
    j = np.arange(G)
    valid = j[None, :] < cnt[:, None]                      # [C, G]
    pos = starts[:-1, None] + np.minimum(j[None, :],
                                         np.maximum(cnt[:, None] - 1, 0))
    gidx = order[pos]                                      # [C, G]

    Xp = np.asarray(x, np.float32)[gidx]                   # [C, G, S]
    Up = np.asarray(u, np.float32)[gidx]                   # [C, G, A]
    XpT = np.ascontiguousarray(Xp.transpose(0, 2, 1))      # [C, S, G]
    UpT = np.ascontiguousarray(Up.transpose(0, 2, 1))      # [C, A, G]

    WA = np.ascontiguousarray(WA, np.float32)
    WB = np.ascontiguousarray(WB, np.float32)
    adjxx_u8 = np.ascontiguousarray(adj_xx).view(np.uint8).reshape(S // 128, 128, S)
    adjxu_u8 = np.ascontiguousarray(adj_xu).view(np.uint8)

    in_maps = []
    for k in range(N_CORES):
        sl = slice(k * CP, (k + 1) * CP)
        in_maps.append({
            "wa": WA[sl].reshape(CP, S // 128, 128, S),
            "wb": WB[sl],
            "xt": XpT[sl].reshape(CP, S // 128, 128, G),
            "ut": UpT[sl],
            "adj_xx": adjxx_u8,
            "adj_xu": adjxu_u8,
        })

    if _trace:
        _install_profile_shim()
    nc = _build_program(CP, S, A, G)
    res = run_bass_kernel_spmd(nc, in_maps, core_ids=list(range(N_CORES)),
                               trace=_trace)

    Out_all = np.concatenate([r["out"] for r in res.results], axis=0)  # [C,G,S]
    out_full = np.zeros((B, S), np.float32)
    out_full[gidx[valid]] = Out_all[valid]

    if _trace:
        return out_full, res
    return out_full
